# revision 23
# baseline (speedup 1.0000x reference)
"""Distributed GaussianBasis rasterization on 8 NeuronCores via a Bass/Tile kernel.

Strategy (per the sharding hint): shard the H*W pixel dimension across the 8
cores — each core rasterizes a 32-row slab. Within a core, the slab is split
into 4 bands of 8 rows. The host culls gaussians per band (y-bbox test against
the alpha>=1/255 support ellipse; measured max ~75 per band, padded to 128
slots) and ships per-band coefficient tables:

  w[g,p]   = A[g] - Q[g]*px(p) - T[g]*py(p)    (complete-the-square axis)
  sigma    = w^2 + b[g,row]                     b = 0.5*q*dy^2
  alpha    = opa * exp(-sigma) = rho[g,row] * exp(-w^2)

On device, per band:
  PE:   w = coefT(9 x 128, 3-level bf16 hi/lo split for exactness) @ basis(9 x 2048)
  ACT:  w2 = Square(w)   [PSUM->SBUF];  E = Exp(-w2)  -> bf16
  DVE:  wgt[:, row] = E * rho[:, row]   (per-partition scalar, 4x mode)
  PE:   out[150, 2048] += feats[128, 150]^T @ wgt     (bf16, fp32 accum)
  DVE:  copy PSUM -> SBUF;  DMA -> DRAM

The alpha>=1/255 threshold and 0.999 clamp are dropped (host-verified rel err
3.9e-3 vs the 2e-2 gate); out-of-band gaussians are exactly zero in the
reference, so culling is lossless.
"""
import numpy as np
import ml_dtypes
from contextlib import ExitStack

H = W = 256
N = 1024
M = 50
M3 = 3 * M               # 150
NCORES = 8
ROWS_PER_CORE = H // NCORES   # 32
BAND_ROWS = 8
NBANDS = ROWS_PER_CORE // BAND_ROWS   # 4 bands per core
PIX_BAND = BAND_ROWS * W              # 2048
KPAD = 128
NLEV = 3
KDIM = 3 * NLEV                       # 9 matmul contraction rows
M3P = 160                             # m1 padded to 32 output rows

BF16 = ml_dtypes.bfloat16

_EXEC = None     # cached (sharded_jit_fn, in_names, out_names, out_avals, n_params)


def _build_nc():
    import concourse.tile as tile
    from concourse import bacc, mybir

    bf16 = mybir.dt.bfloat16
    f32 = mybir.dt.float32

    nc = bacc.Bacc("TRN2", target_bir_lowering=False, debug=False,
                   enable_asserts=False)
    # basis | coef packed on the free dim (both [KDIM, *] bf16)
    bc_d = nc.dram_tensor("bc", (KDIM, NBANDS * PIX_BAND + NBANDS * KPAD),
                          bf16, kind="ExternalInput").ap()
    # feats | rho packed on the free dim ([KPAD, *] bf16; rho is f32 bitcast)
    fr_d = nc.dram_tensor("fr", (KPAD, NBANDS * M3P + 2 * NBANDS * BAND_ROWS),
                          bf16, kind="ExternalInput").ap()
    out_d = nc.dram_tensor("out", (M3, ROWS_PER_CORE * W), f32,
                           kind="ExternalOutput").ap()

    Square = mybir.ActivationFunctionType.Square
    Exp = mybir.ActivationFunctionType.Exp

    with ExitStack() as ctx:
        tc = ctx.enter_context(tile.TileContext(nc))
        const = ctx.enter_context(tc.tile_pool(name="const", bufs=1))
        wk = ctx.enter_context(tc.tile_pool(name="wk", bufs=3))
        ob = ctx.enter_context(tc.tile_pool(name="ob", bufs=2))
        psw = ctx.enter_context(tc.tile_pool(name="psw", bufs=2, space="PSUM"))
        psm0 = ctx.enter_context(tc.tile_pool(name="psm0", bufs=2, space="PSUM"))
        psm1 = ctx.enter_context(tc.tile_pool(name="psm1", bufs=1, space="PSUM"))

        sb_bc = const.tile([KDIM, NBANDS * PIX_BAND + NBANDS * KPAD], bf16)
        nc.scalar.dma_start(sb_bc[:], bc_d)
        sb_fr = const.tile([KPAD, NBANDS * M3P + 2 * NBANDS * BAND_ROWS], bf16)
        nc.sync.dma_start(sb_fr[:], fr_d)
        sb_basis = sb_bc[:, 0:NBANDS * PIX_BAND]
        sb_coef = sb_bc[:, NBANDS * PIX_BAND:]
        sb_feats = sb_fr[:, 0:NBANDS * M3P]
        sb_rho = sb_fr[:, NBANDS * M3P:].bitcast(f32)

        # PE warm-up: keep the tensor engine busy while input DMAs are in
        # flight so the HAM clock gate releases (1.2 -> 2.4 GHz) before the
        # real matmuls start. Results go to a scratch PSUM slot, never read.
        scratch = const.tile([128, 512], bf16)
        nc.vector.memset(scratch[:], 0.0)
        for _ in range(14):
            pmw = psm0.tile([128, 512], f32, tag="pm0")
            nc.tensor.matmul(pmw[:], scratch[:, 0:128], scratch[:],
                             start=True, stop=True)

        for b in range(NBANDS):
            lhs_m0 = sb_feats[:, b * M3P: b * M3P + 128]
            lhs_m1 = sb_feats[:, b * M3P + 128: b * M3P + 160]
            out_m1 = ob.tile([64, 1024], f32, tag="om1")
            pm1 = psm1.tile([64, 1024], f32, tag="pm1")

            for u in range(2):            # 1024-pixel units
                base = b * PIX_BAND + u * 1024
                ps_w = psw.tile([128, 1024], f32, tag="w")
                for h in range(2):
                    nc.tensor.matmul(
                        ps_w[:, h * 512:(h + 1) * 512],
                        sb_coef[:, b * KPAD:(b + 1) * KPAD],
                        sb_basis[:, base + h * 512: base + (h + 1) * 512],
                        start=True, stop=True)
                # filler matmuls into the pm0 slot rotation: they execute
                # while PE would otherwise stall on the wgt producers, which
                # keeps the HAM activity window busy (PE stays at 2.4 GHz)
                for _ in range(5):
                    pmf = psm0.tile([128, 512], f32, tag="pm0")
                    nc.tensor.matmul(pmf[:], scratch[:, 0:128], scratch[:],
                                     start=True, stop=True)
                w2 = wk.tile([128, 1024], f32, tag="w2")
                nc.scalar.activation(w2[:], ps_w[:], Square)
                E = wk.tile([128, 1024], bf16, tag="E")
                nc.scalar.activation(E[:], w2[:], Exp, scale=-1.0)
                wgt = wk.tile([128, 1024], bf16, tag="wgt")
                for r in range(4):
                    row = b * BAND_ROWS + u * 4 + r
                    nc.vector.tensor_scalar_mul(
                        wgt[:, r * 256:(r + 1) * 256],
                        E[:, r * 256:(r + 1) * 256],
                        sb_rho[:, row:row + 1])
                out_m0 = ob.tile([128, 1024], f32, tag="om0")
                for h in range(2):
                    pm0 = psm0.tile([128, 512], f32, tag="pm0")
                    nc.tensor.matmul(
                        pm0[:], lhs_m0,
                        wgt[:, h * 512:(h + 1) * 512],
                        start=True, stop=True)
                    if h == 0:
                        nc.scalar.copy(out_m0[:, 0:512], pm0[:])
                    else:
                        nc.vector.tensor_copy(
                            out_m0[:, 512:1024], pm0[:])
                # m1 (output rows 128:150) pair-packed into one PSUM tile:
                # unit 0 -> partitions 0:22, unit 1 -> partitions 32:54
                for h in range(2):
                    nc.tensor.matmul(
                        pm1[u * 32:(u + 1) * 32, h * 512:(h + 1) * 512],
                        lhs_m1,
                        wgt[:, h * 512:(h + 1) * 512],
                        start=True, stop=True,
                        tile_position=(0, u * 32))
                nc.sync.dma_start(
                    out_d[0:128, base: base + 1024], out_m0[:])
            nc.vector.tensor_copy(out_m1[:], pm1[:])
            nc.sync.dma_start(
                out_d[128:M3, b * PIX_BAND: b * PIX_BAND + 1024],
                out_m1[0:22, :])
            nc.sync.dma_start(
                out_d[128:M3, b * PIX_BAND + 1024:(b + 1) * PIX_BAND],
                out_m1[32:54, :])
    nc.finalize()
    return nc


def _get_executor():
    global _EXEC
    if _EXEC is not None:
        return _EXEC
    import jax
    import jax.core
    from jax.experimental.shard_map import shard_map
    from jax.sharding import Mesh, PartitionSpec
    from concourse import mybir
    from concourse.bass2jax import (_bass_exec_p, install_neuronx_cc_hook,
                                    partition_id_tensor)

    install_neuronx_cc_hook()
    nc = _build_nc()
    partition_name = (nc.partition_id_tensor.name
                      if nc.partition_id_tensor else None)

    in_names, out_names, out_avals = [], [], []
    for alloc in nc.m.functions[0].allocations:
        if not isinstance(alloc, mybir.MemoryLocationSet):
            continue
        name = alloc.memorylocations[0].name
        if alloc.kind == "ExternalInput":
            if name != partition_name:
                in_names.append(name)
        elif alloc.kind == "ExternalOutput":
            out_names.append(name)
            out_avals.append(jax.core.ShapedArray(
                tuple(alloc.tensor_shape), mybir.dt.np(alloc.dtype)))
    n_params = len(in_names)
    n_outs = len(out_names)
    all_names = in_names + out_names
    if partition_name is not None:
        all_names = all_names + [partition_name]
    donate = tuple(range(n_params, n_params + n_outs))

    def _body(*args):
        operands = list(args)
        if partition_name is not None:
            operands.append(partition_id_tensor())
        outs = _bass_exec_p.bind(
            *operands,
            out_avals=tuple(out_avals),
            in_names=tuple(all_names),
            out_names=tuple(out_names),
            lowering_input_output_aliases=(),
            sim_require_finite=False,
            sim_require_nnan=False,
            nc=nc,
        )
        return tuple(outs)

    devices = jax.devices()[:NCORES]
    mesh = Mesh(np.asarray(devices), ("core",))
    in_specs = (PartitionSpec("core"),) * (n_params + n_outs)
    out_specs = (PartitionSpec("core"),) * n_outs
    sharded = jax.jit(
        shard_map(_body, mesh=mesh, in_specs=in_specs, out_specs=out_specs,
                  check_rep=False),
        donate_argnums=donate, keep_unused=True)
    _EXEC = (sharded, in_names, out_names, out_avals, n_params)
    return _EXEC


def _split3(v):
    """3-level bf16 hi/lo decomposition of an f64 vector (exact to ~2^-27)."""
    parts = []
    rem = v.copy()
    for _ in range(NLEV):
        h = rem.astype(np.float32).astype(BF16)
        parts.append(h)
        rem = rem - h.astype(np.float64)
    return parts


def _host_prepare(xyz_raw, cholesky_raw, opacity, feats_mn3):
    """Build the per-core input tensors. feats_mn3: [M, N, 3] float32."""
    xy = np.tanh(xyz_raw.astype(np.float64))
    chol = cholesky_raw.astype(np.float64) + np.array([0.5, 0.0, 0.5])
    l1, l2, l3 = chol[:, 0], chol[:, 1], chol[:, 2]
    a_ = l1 * l1
    b_ = l1 * l2
    c_ = l2 * l2 + l3 * l3
    det = a_ * c_ - b_ * b_
    c1 = c_ / det
    c2 = -b_ / det
    r = c2 / c1
    q = 1.0 / c_                      # Schur complement c3 - c2^2/c1 == 1/cov_yy
    cx = 0.5 * ((xy[:, 0] + 1.0) * W - 1.0)
    cy = 0.5 * ((xy[:, 1] + 1.0) * H - 1.0)
    s = np.sqrt(0.5 * c1)
    A = s * (cx + r * cy)
    Q = s
    T = s * r
    opa = opacity[:, 0].astype(np.float64)
    tau = np.log(255.0 * np.maximum(opa, 1e-300))
    visible = tau > 0
    hy = np.sqrt(2.0 * np.maximum(tau, 0.0) * c_)

    A3, Q3, T3 = _split3(A), _split3(Q), _split3(T)
    feats_nm = np.ascontiguousarray(
        feats_mn3.transpose(1, 0, 2).reshape(N, M3)).astype(BF16)

    px = np.arange(W, dtype=np.float64)

    in_maps = []
    for core in range(NCORES):
        basis = np.zeros((KDIM, NBANDS * PIX_BAND), np.float32)
        coef = np.zeros((KDIM, NBANDS * KPAD), BF16)
        rho = np.zeros((KPAD, NBANDS * BAND_ROWS), np.float32)
        feats = np.zeros((KPAD, NBANDS * M3P), BF16)
        for b in range(NBANDS):
            y0 = core * ROWS_PER_CORE + b * BAND_ROWS
            ys = np.arange(y0, y0 + BAND_ROWS, dtype=np.float64)
            idx = np.nonzero(visible & (cy + hy >= y0 - 0.5)
                             & (cy - hy <= y0 + BAND_ROWS - 1 + 0.5))[0]
            if len(idx) > KPAD:
                # keep the strongest by peak row weight (never triggers for
                # the reference distribution; safety valve only)
                dy2 = (cy[idx][:, None] - ys[None, :]) ** 2
                peak = (opa[idx][:, None]
                        * np.exp(-0.5 * q[idx][:, None] * dy2)).max(axis=1)
                idx = idx[np.argsort(-peak)[:KPAD]]
            k = len(idx)
            # basis rows: [1, -px, -py] x 3 levels (all exactly bf16)
            for lv in range(NLEV):
                basis[3 * lv + 0, b * PIX_BAND:(b + 1) * PIX_BAND] = 1.0
                basis[3 * lv + 1, b * PIX_BAND:(b + 1) * PIX_BAND] = \
                    -np.tile(px, BAND_ROWS)
                basis[3 * lv + 2, b * PIX_BAND:(b + 1) * PIX_BAND] = \
                    -np.repeat(ys, W)
                coef[3 * lv + 0, b * KPAD: b * KPAD + k] = A3[lv][idx]
                coef[3 * lv + 1, b * KPAD: b * KPAD + k] = Q3[lv][idx]
                coef[3 * lv + 2, b * KPAD: b * KPAD + k] = T3[lv][idx]
            dy = cy[idx][:, None] - ys[None, :]
            rho[:k, b * BAND_ROWS:(b + 1) * BAND_ROWS] = \
                (opa[idx][:, None] * np.exp(-0.5 * q[idx][:, None] * dy * dy))
            feats[:k, b * M3P: b * M3P + M3] = feats_nm[idx]
        bc = np.concatenate([basis.astype(BF16), coef], axis=1)
        rho_as_bf16 = np.ascontiguousarray(rho).view(np.uint8).reshape(
            KPAD, -1).view(BF16)
        fr = np.concatenate([feats, rho_as_bf16], axis=1)
        in_maps.append({"bc": bc, "fr": fr})
    return in_maps


def kernel(xyz_raw, cholesky_raw, opacity, features_dc, cluster_id):
    feats_mn3 = np.asarray(features_dc)[int(cluster_id)].astype(np.float32)
    in_maps = _host_prepare(np.asarray(xyz_raw, np.float32),
                            np.asarray(cholesky_raw, np.float32),
                            np.asarray(opacity, np.float32),
                            feats_mn3)
    sharded, in_names, out_names, out_avals, n_params = _get_executor()

    concat_in = [np.concatenate([in_maps[c][name] for c in range(NCORES)],
                                axis=0) for name in in_names]
    concat_zeros = [np.zeros((NCORES * av.shape[0], *av.shape[1:]), av.dtype)
                    for av in out_avals]
    out_arrs = sharded(*concat_in, *concat_zeros)
    full = np.asarray(out_arrs[0]).reshape(NCORES, M3, ROWS_PER_CORE * W)
    flat = np.concatenate([full[c] for c in range(NCORES)], axis=1)
    return flat.reshape(M, 3, H, W).astype(np.float32)


# revision 24
# speedup vs baseline: 1.0372x; 1.0372x over previous
"""Distributed GaussianBasis rasterization on 8 NeuronCores via a Bass/Tile kernel.

Strategy (per the sharding hint): shard the H*W pixel dimension across the 8
cores — each core rasterizes a 32-row slab. Within a core, the slab is split
into 4 bands of 8 rows. The host culls gaussians per band (y-bbox test against
the alpha>=1/255 support ellipse; measured max ~75 per band, padded to 128
slots) and ships per-band coefficient tables:

  w[g,p]   = A[g] - Q[g]*px(p) - T[g]*py(p)    (complete-the-square axis)
  sigma    = w^2 + b[g,row]                     b = 0.5*q*dy^2
  alpha    = opa * exp(-sigma) = rho[g,row] * exp(-w^2)

On device, per band:
  PE:   w = coefT(9 x 128, 3-level bf16 hi/lo split for exactness) @ basis(9 x 2048)
  ACT:  w2 = Square(w)   [PSUM->SBUF];  E = Exp(-w2)  -> bf16
  DVE:  wgt[:, row] = E * rho[:, row]   (per-partition scalar, 4x mode)
  PE:   out[150, 2048] += feats[128, 150]^T @ wgt     (bf16, fp32 accum)
  DVE:  copy PSUM -> SBUF;  DMA -> DRAM

The alpha>=1/255 threshold and 0.999 clamp are dropped (host-verified rel err
3.9e-3 vs the 2e-2 gate); out-of-band gaussians are exactly zero in the
reference, so culling is lossless.
"""
import numpy as np
import ml_dtypes
from contextlib import ExitStack

H = W = 256
N = 1024
M = 50
M3 = 3 * M               # 150
NCORES = 8
ROWS_PER_CORE = H // NCORES   # 32
BAND_ROWS = 8
NBANDS = ROWS_PER_CORE // BAND_ROWS   # 4 bands per core
PIX_BAND = BAND_ROWS * W              # 2048
KPAD = 128
NLEV = 3
KDIM = 3 * NLEV                       # 9 matmul contraction rows
M3P = 160                             # m1 padded to 32 output rows

BF16 = ml_dtypes.bfloat16

_EXEC = None     # cached (sharded_jit_fn, in_names, out_names, out_avals, n_params)


def _build_nc():
    import concourse.tile as tile
    from concourse import bacc, mybir

    bf16 = mybir.dt.bfloat16
    f32 = mybir.dt.float32

    nc = bacc.Bacc("TRN2", target_bir_lowering=False, debug=False,
                   enable_asserts=False)
    # basis | coef packed on the free dim (both [KDIM, *] bf16)
    bc_d = nc.dram_tensor("bc", (KDIM, NBANDS * PIX_BAND + NBANDS * KPAD),
                          bf16, kind="ExternalInput").ap()
    # feats | rho packed on the free dim ([KPAD, *] bf16; rho is f32 bitcast)
    fr_d = nc.dram_tensor("fr", (KPAD, NBANDS * M3P + 2 * NBANDS * BAND_ROWS),
                          bf16, kind="ExternalInput").ap()
    out_d = nc.dram_tensor("out", (M3, ROWS_PER_CORE * W), f32,
                           kind="ExternalOutput").ap()

    Square = mybir.ActivationFunctionType.Square
    Exp = mybir.ActivationFunctionType.Exp

    with ExitStack() as ctx:
        tc = ctx.enter_context(tile.TileContext(nc))
        const = ctx.enter_context(tc.tile_pool(name="const", bufs=1))
        wk = ctx.enter_context(tc.tile_pool(name="wk", bufs=3))
        ob = ctx.enter_context(tc.tile_pool(name="ob", bufs=2))
        psw = ctx.enter_context(tc.tile_pool(name="psw", bufs=2, space="PSUM"))
        psm0 = ctx.enter_context(tc.tile_pool(name="psm0", bufs=2, space="PSUM"))
        psm1 = ctx.enter_context(tc.tile_pool(name="psm1", bufs=1, space="PSUM"))

        sb_bc = const.tile([KDIM, NBANDS * PIX_BAND + NBANDS * KPAD], bf16)
        nc.sync.dma_start(sb_bc[:], bc_d)
        sb_fr = const.tile([KPAD, NBANDS * M3P + 2 * NBANDS * BAND_ROWS], bf16)
        nc.sync.dma_start(sb_fr[:], fr_d)
        sb_basis = sb_bc[:, 0:NBANDS * PIX_BAND]
        sb_coef = sb_bc[:, NBANDS * PIX_BAND:]
        sb_feats = sb_fr[:, 0:NBANDS * M3P]
        sb_rho = sb_fr[:, NBANDS * M3P:].bitcast(f32)

        # PE warm-up: keep the tensor engine busy while input DMAs are in
        # flight so the HAM clock gate releases (1.2 -> 2.4 GHz) before the
        # real matmuls start. Results go to a scratch PSUM slot, never read.
        scratch = const.tile([128, 512], bf16)
        nc.vector.memset(scratch[:], 0.0)
        for _ in range(14):
            pmw = psm0.tile([128, 512], f32, tag="pm0")
            nc.tensor.matmul(pmw[:], scratch[:, 0:128], scratch[:],
                             start=True, stop=True)

        for b in range(NBANDS):
            lhs_m0 = sb_feats[:, b * M3P: b * M3P + 128]
            lhs_m1 = sb_feats[:, b * M3P + 128: b * M3P + 160]
            out_m1 = ob.tile([64, 1024], bf16, tag="om1")
            pm1 = psm1.tile([64, 1024], f32, tag="pm1")

            for u in range(2):            # 1024-pixel units
                base = b * PIX_BAND + u * 1024
                ps_w = psw.tile([128, 1024], f32, tag="w")
                for h in range(2):
                    nc.tensor.matmul(
                        ps_w[:, h * 512:(h + 1) * 512],
                        sb_coef[:, b * KPAD:(b + 1) * KPAD],
                        sb_basis[:, base + h * 512: base + (h + 1) * 512],
                        start=True, stop=True)
                # filler matmuls into the pm0 slot rotation: they execute
                # while PE would otherwise stall on the wgt producers, which
                # keeps the HAM activity window busy (PE stays at 2.4 GHz)
                for _ in range(4):
                    pmf = psm0.tile([128, 512], f32, tag="pm0")
                    nc.tensor.matmul(pmf[:], scratch[:, 0:128], scratch[:],
                                     start=True, stop=True)
                w2 = wk.tile([128, 1024], f32, tag="w2")
                nc.scalar.activation(w2[:], ps_w[:], Square)
                E = wk.tile([128, 1024], bf16, tag="E")
                nc.scalar.activation(E[:], w2[:], Exp, scale=-1.0)
                wgt = wk.tile([128, 1024], bf16, tag="wgt")
                for r in range(4):
                    row = b * BAND_ROWS + u * 4 + r
                    nc.vector.tensor_scalar_mul(
                        wgt[:, r * 256:(r + 1) * 256],
                        E[:, r * 256:(r + 1) * 256],
                        sb_rho[:, row:row + 1])
                out_m0 = ob.tile([128, 1024], bf16, tag="om0")
                for h in range(2):
                    pm0 = psm0.tile([128, 512], f32, tag="pm0")
                    nc.tensor.matmul(
                        pm0[:], lhs_m0,
                        wgt[:, h * 512:(h + 1) * 512],
                        start=True, stop=True)
                    if h == 0:
                        nc.scalar.copy(out_m0[:, 0:512], pm0[:])
                    else:
                        nc.vector.tensor_copy(
                            out_m0[:, 512:1024], pm0[:])
                # m1 (output rows 128:150) pair-packed into one PSUM tile:
                # unit 0 -> partitions 0:22, unit 1 -> partitions 32:54
                for h in range(2):
                    nc.tensor.matmul(
                        pm1[u * 32:(u + 1) * 32, h * 512:(h + 1) * 512],
                        lhs_m1,
                        wgt[:, h * 512:(h + 1) * 512],
                        start=True, stop=True,
                        tile_position=(0, u * 32))
                nc.gpsimd.dma_start(
                    out_d[0:128, base: base + 1024], out_m0[:])
            nc.vector.tensor_copy(out_m1[:], pm1[:])
            nc.gpsimd.dma_start(
                out_d[128:M3, b * PIX_BAND: b * PIX_BAND + 1024],
                out_m1[0:22, :])
            nc.gpsimd.dma_start(
                out_d[128:M3, b * PIX_BAND + 1024:(b + 1) * PIX_BAND],
                out_m1[32:54, :])
    nc.finalize()
    return nc


def _get_executor():
    global _EXEC
    if _EXEC is not None:
        return _EXEC
    import jax
    import jax.core
    from jax.experimental.shard_map import shard_map
    from jax.sharding import Mesh, PartitionSpec
    from concourse import mybir
    from concourse.bass2jax import (_bass_exec_p, install_neuronx_cc_hook,
                                    partition_id_tensor)

    install_neuronx_cc_hook()
    nc = _build_nc()
    partition_name = (nc.partition_id_tensor.name
                      if nc.partition_id_tensor else None)

    in_names, out_names, out_avals = [], [], []
    for alloc in nc.m.functions[0].allocations:
        if not isinstance(alloc, mybir.MemoryLocationSet):
            continue
        name = alloc.memorylocations[0].name
        if alloc.kind == "ExternalInput":
            if name != partition_name:
                in_names.append(name)
        elif alloc.kind == "ExternalOutput":
            out_names.append(name)
            out_avals.append(jax.core.ShapedArray(
                tuple(alloc.tensor_shape), mybir.dt.np(alloc.dtype)))
    n_params = len(in_names)
    n_outs = len(out_names)
    all_names = in_names + out_names
    if partition_name is not None:
        all_names = all_names + [partition_name]
    donate = tuple(range(n_params, n_params + n_outs))

    def _body(*args):
        operands = list(args)
        if partition_name is not None:
            operands.append(partition_id_tensor())
        outs = _bass_exec_p.bind(
            *operands,
            out_avals=tuple(out_avals),
            in_names=tuple(all_names),
            out_names=tuple(out_names),
            lowering_input_output_aliases=(),
            sim_require_finite=False,
            sim_require_nnan=False,
            nc=nc,
        )
        return tuple(outs)

    devices = jax.devices()[:NCORES]
    mesh = Mesh(np.asarray(devices), ("core",))
    in_specs = (PartitionSpec("core"),) * (n_params + n_outs)
    out_specs = (PartitionSpec("core"),) * n_outs
    sharded = jax.jit(
        shard_map(_body, mesh=mesh, in_specs=in_specs, out_specs=out_specs,
                  check_rep=False),
        donate_argnums=donate, keep_unused=True)
    _EXEC = (sharded, in_names, out_names, out_avals, n_params)
    return _EXEC


def _split3(v):
    """3-level bf16 hi/lo decomposition of an f64 vector (exact to ~2^-27)."""
    parts = []
    rem = v.copy()
    for _ in range(NLEV):
        h = rem.astype(np.float32).astype(BF16)
        parts.append(h)
        rem = rem - h.astype(np.float64)
    return parts


def _host_prepare(xyz_raw, cholesky_raw, opacity, feats_mn3):
    """Build the per-core input tensors. feats_mn3: [M, N, 3] float32."""
    xy = np.tanh(xyz_raw.astype(np.float64))
    chol = cholesky_raw.astype(np.float64) + np.array([0.5, 0.0, 0.5])
    l1, l2, l3 = chol[:, 0], chol[:, 1], chol[:, 2]
    a_ = l1 * l1
    b_ = l1 * l2
    c_ = l2 * l2 + l3 * l3
    det = a_ * c_ - b_ * b_
    c1 = c_ / det
    c2 = -b_ / det
    r = c2 / c1
    q = 1.0 / c_                      # Schur complement c3 - c2^2/c1 == 1/cov_yy
    cx = 0.5 * ((xy[:, 0] + 1.0) * W - 1.0)
    cy = 0.5 * ((xy[:, 1] + 1.0) * H - 1.0)
    s = np.sqrt(0.5 * c1)
    A = s * (cx + r * cy)
    Q = s
    T = s * r
    opa = opacity[:, 0].astype(np.float64)
    tau = np.log(255.0 * np.maximum(opa, 1e-300))
    visible = tau > 0
    hy = np.sqrt(2.0 * np.maximum(tau, 0.0) * c_)

    A3, Q3, T3 = _split3(A), _split3(Q), _split3(T)
    feats_nm = np.ascontiguousarray(
        feats_mn3.transpose(1, 0, 2).reshape(N, M3)).astype(BF16)

    px = np.arange(W, dtype=np.float64)

    in_maps = []
    for core in range(NCORES):
        basis = np.zeros((KDIM, NBANDS * PIX_BAND), np.float32)
        coef = np.zeros((KDIM, NBANDS * KPAD), BF16)
        rho = np.zeros((KPAD, NBANDS * BAND_ROWS), np.float32)
        feats = np.zeros((KPAD, NBANDS * M3P), BF16)
        for b in range(NBANDS):
            y0 = core * ROWS_PER_CORE + b * BAND_ROWS
            ys = np.arange(y0, y0 + BAND_ROWS, dtype=np.float64)
            idx = np.nonzero(visible & (cy + hy >= y0 - 0.5)
                             & (cy - hy <= y0 + BAND_ROWS - 1 + 0.5))[0]
            if len(idx) > KPAD:
                # keep the strongest by peak row weight (never triggers for
                # the reference distribution; safety valve only)
                dy2 = (cy[idx][:, None] - ys[None, :]) ** 2
                peak = (opa[idx][:, None]
                        * np.exp(-0.5 * q[idx][:, None] * dy2)).max(axis=1)
                idx = idx[np.argsort(-peak)[:KPAD]]
            k = len(idx)
            # basis rows: [1, -px, -py] x 3 levels (all exactly bf16)
            for lv in range(NLEV):
                basis[3 * lv + 0, b * PIX_BAND:(b + 1) * PIX_BAND] = 1.0
                basis[3 * lv + 1, b * PIX_BAND:(b + 1) * PIX_BAND] = \
                    -np.tile(px, BAND_ROWS)
                basis[3 * lv + 2, b * PIX_BAND:(b + 1) * PIX_BAND] = \
                    -np.repeat(ys, W)
                coef[3 * lv + 0, b * KPAD: b * KPAD + k] = A3[lv][idx]
                coef[3 * lv + 1, b * KPAD: b * KPAD + k] = Q3[lv][idx]
                coef[3 * lv + 2, b * KPAD: b * KPAD + k] = T3[lv][idx]
            dy = cy[idx][:, None] - ys[None, :]
            rho[:k, b * BAND_ROWS:(b + 1) * BAND_ROWS] = \
                (opa[idx][:, None] * np.exp(-0.5 * q[idx][:, None] * dy * dy))
            feats[:k, b * M3P: b * M3P + M3] = feats_nm[idx]
        bc = np.concatenate([basis.astype(BF16), coef], axis=1)
        rho_as_bf16 = np.ascontiguousarray(rho).view(np.uint8).reshape(
            KPAD, -1).view(BF16)
        fr = np.concatenate([feats, rho_as_bf16], axis=1)
        in_maps.append({"bc": bc, "fr": fr})
    return in_maps


def kernel(xyz_raw, cholesky_raw, opacity, features_dc, cluster_id):
    feats_mn3 = np.asarray(features_dc)[int(cluster_id)].astype(np.float32)
    in_maps = _host_prepare(np.asarray(xyz_raw, np.float32),
                            np.asarray(cholesky_raw, np.float32),
                            np.asarray(opacity, np.float32),
                            feats_mn3)
    sharded, in_names, out_names, out_avals, n_params = _get_executor()

    concat_in = [np.concatenate([in_maps[c][name] for c in range(NCORES)],
                                axis=0) for name in in_names]
    concat_zeros = [np.zeros((NCORES * av.shape[0], *av.shape[1:]), av.dtype)
                    for av in out_avals]
    out_arrs = sharded(*concat_in, *concat_zeros)
    full = np.asarray(out_arrs[0]).reshape(NCORES, M3, ROWS_PER_CORE * W)
    flat = np.concatenate([full[c] for c in range(NCORES)], axis=1)
    return flat.reshape(M, 3, H, W).astype(np.float32)
